# revision 2
# baseline (speedup 1.0000x reference)
"""Row-wise cosine-similarity loss (1 - mean(cos)) for N=16384, D=2048 f32.

Data-parallel across 8 NeuronCores: each core gets 2048 rows of both
tensors, computes per-row dot / ||a||^2 / ||b||^2 with fused
multiply-reduce ops (DVE scalar_tensor_tensor + ACT square-with-accum),
then the row cosines; the host sums the 8x[128,16] outputs into the
scalar loss.

The walrus build in this container accepts at most ONE semaphore wait
per instruction; Tile emits several.  _split_multi_waits() post-passes
the BIR and hoists extra waits onto NOPs inserted just before the
offending instruction on the same engine.
"""

import numpy as np

N, D = 16384, 2048
NCORES = 8
NS = N // NCORES  # rows per core
P = 128  # SBUF partitions
T = NS // P  # row-tiles per core (16)
CH = 4  # row-tiles per DMA chunk (4 MiB per tensor per chunk)
NCH = T // CH

_cached_nc = None


def _split_multi_waits(nc):
    """Walrus here supports one sem-wait per instruction; split extras
    onto NOPs inserted immediately before, on the same engine."""
    import concourse.mybir as mybir

    n = 0
    for f in nc.m.functions:
        for bb in f.blocks:
            insts = bb.instructions
            out = []
            changed = False
            for ins in insts:
                si = getattr(ins, "sync_info", None)
                ow = list(si.on_wait) if si is not None and si.on_wait else []
                if len(ow) > 1:
                    changed = True
                    for w in ow[:-1]:
                        n += 1
                        out.append(
                            mybir.InstNoOp(
                                name=f"{ins.name}-wsplit{n}",
                                engine=ins.engine,
                                bass_nofuse=True,
                                sync_info=mybir.SyncInfo(
                                    on_wait=[w], on_update=[]
                                ),
                            )
                        )
                    si.on_wait = [ow[-1]]
                out.append(ins)
            if changed:
                bb.instructions = out
    return n


def _build():
    import concourse.bass as bass
    import concourse.mybir as mybir
    import concourse.tile as tile

    f32 = mybir.dt.float32
    Alu = mybir.AluOpType
    Act = mybir.ActivationFunctionType

    nc = bass.Bass("TRN2", target_bir_lowering=False)
    a = nc.dram_tensor("ehr", [NS, D], f32, kind="ExternalInput")
    b = nc.dram_tensor("cxr", [NS, D], f32, kind="ExternalInput")
    out = nc.dram_tensor("cos", [P, T], f32, kind="ExternalOutput")

    # chunk c covers rows [c*CH*P, (c+1)*CH*P); tile t inside covers 128 rows
    av = a.rearrange("(c t p) d -> c p t d", t=CH, p=P)
    bv = b.rearrange("(c t p) d -> c p t d", t=CH, p=P)

    def dot_dve(dst, x, y, scr):
        # scr = x*y ; dst = sum(x*y) along free dim
        nc.vector.scalar_tensor_tensor(
            out=scr,
            in0=x,
            scalar=1.0,
            in1=y,
            op0=Alu.mult,
            op1=Alu.mult,
            accum_out=dst,
        )

    with tile.TileContext(nc) as tc:
        with (
            tc.tile_pool(name="apool", bufs=2) as apool,
            tc.tile_pool(name="bpool", bufs=2) as bpool,
            tc.tile_pool(name="singles", bufs=1) as singles,
            tc.tile_pool(name="small", bufs=2) as small,
        ):
            dot_buf = singles.tile([P, T], f32, tag="dot")
            na_buf = singles.tile([P, T], f32, tag="na")
            nb_buf = singles.tile([P, T], f32, tag="nb")
            cos_buf = singles.tile([P, T], f32, tag="cos")
            scr_dve = singles.tile([P, D], f32, tag="scr_dve")
            scr_act = singles.tile([P, D], f32, tag="scr_act")

            for c in range(NCH):
                at = apool.tile([P, CH, D], f32, tag="a")
                bt = bpool.tile([P, CH, D], f32, tag="b")
                nc.sync.dma_start(out=at, in_=av[c])
                nc.sync.dma_start(out=bt, in_=bv[c])
                for t in range(CH):
                    g = c * CH + t
                    dot_dve(dot_buf[:, g : g + 1], at[:, t, :], bt[:, t, :], scr_dve)
                    # ||a||^2: usually ACT (square w/ accum); every 3rd tile
                    # on DVE to balance engine load under the DMA roofline.
                    if g % 3 == 0:
                        dot_dve(na_buf[:, g : g + 1], at[:, t, :], at[:, t, :], scr_dve)
                    else:
                        nc.scalar.activation(
                            out=scr_act,
                            in_=at[:, t, :],
                            func=Act.Square,
                            accum_out=na_buf[:, g : g + 1],
                        )
                    nc.scalar.activation(
                        out=scr_act,
                        in_=bt[:, t, :],
                        func=Act.Square,
                        accum_out=nb_buf[:, g : g + 1],
                    )

            # cos = dot / sqrt(na*nb), batched over all T columns
            prod = small.tile([P, T], f32, tag="prod")
            nc.vector.tensor_mul(prod, na_buf, nb_buf)
            rs = small.tile([P, T], f32, tag="rs")
            nc.scalar.sqrt(rs, prod)
            rr = small.tile([P, T], f32, tag="rr")
            nc.vector.reciprocal(rr, rs)
            nc.vector.tensor_mul(cos_buf, dot_buf, rr)
            nc.sync.dma_start(out=out[:], in_=cos_buf)

    _split_multi_waits(nc)
    return nc


def _get_nc():
    global _cached_nc
    if _cached_nc is None:
        _cached_nc = _build()
    return _cached_nc


def _run(in_maps, **kwargs):
    from concourse.bass_utils import run_bass_kernel_spmd

    return run_bass_kernel_spmd(_get_nc(), in_maps, core_ids=list(range(NCORES)), **kwargs)


def _make_in_maps(cxr, ehr):
    cxr = np.ascontiguousarray(np.asarray(cxr), dtype=np.float32)
    ehr = np.ascontiguousarray(np.asarray(ehr), dtype=np.float32)
    return [
        {
            "cxr": cxr[i * NS : (i + 1) * NS],
            "ehr": ehr[i * NS : (i + 1) * NS],
        }
        for i in range(NCORES)
    ]


def _combine(results):
    cos = np.stack([r["cos"] for r in results])  # [8, 128, T]
    return np.float32(1.0 - cos.astype(np.float64).mean())


def kernel(cxr, ehr):
    res = _run(_make_in_maps(cxr, ehr))
    return _combine(res.results)
